# revision 1
# baseline (speedup 1.0000x reference)
"""Trainium2 Bass kernel for causal attention + proj + causal_features.

Problem shapes: x [2, 2048, 1024], H=16 heads, HD=64.
Strategy (8 NeuronCores):
  - Head-parallel attention: core i computes QKV + attention for heads {2i, 2i+1}
    over both batches, everything in transposed [channel, token] layout so the
    contraction dim sits on SBUF partitions.
  - Scores computed as S^T[k, q] = K @ Q^T (contraction d=64); exp on ScalarE
    (scale=1/8 folded in); causal mask applied only on diagonal-band tiles via
    a precomputed 0/1 multiplier; upper-triangle tiles skipped entirely.
  - attn @ V via lhsT = [V | 1] (ones column) so the softmax denominators fall
    out of the same matmul as row 64 of the PSUM accumulator.
  - AllToAll reshards attention output from head-parallel to token-parallel;
    each core then computes proj + bias and causal_features for its 512-token
    slice. Outputs returned transposed [1024, 512] and re-assembled on host.
  - All matmuls run in float32r (full PE rate at free dim >= 256, ~1e-4 rel err).
"""

import numpy as np
import ml_dtypes  # noqa: F401  (registers bfloat16 with numpy)
import concourse.bass as bass
import concourse.mybir as mybir
import concourse.tile as tile
from concourse import bacc
from concourse.bass_utils import run_bass_kernel_spmd

B, N, C, H, HD = 2, 2048, 1024, 16, 64
NCORES = 8
TOK = B * N            # 4096 global tokens
TPC = TOK // NCORES    # 512 tokens per core (output slice)
QC = 512               # q chunk width
KTILE = 128            # k tile height
NKT = N // KTILE       # 16 k tiles per batch
NQC = N // QC          # 4 q chunks per batch
f32 = mybir.dt.float32
f32r = mybir.dt.float32r
bf16 = mybir.dt.bfloat16
AF = mybir.ActivationFunctionType
ALU = mybir.AluOpType

_CACHE = {}


def _build_program(fake_a2a=False, reps=1, phase=99, av_delay=False, pe_norm=False):
    nc = bacc.Bacc("TRN2", target_bir_lowering=False, debug=False, num_devices=NCORES)

    xT_d = nc.dram_tensor("xT", [C, TOK], f32r, kind="ExternalInput")
    wqkv_d = nc.dram_tensor("wqkv", [128, 3 * 8 * 128], f32r, kind="ExternalInput")
    wp_d = nc.dram_tensor("wp", [8, 128, 8 * 128], f32r, kind="ExternalInput")
    wc_d = nc.dram_tensor("wc", [8, 128, 8 * 128], f32r, kind="ExternalInput")
    bias_d = nc.dram_tensor("bias", [128, 16], f32, kind="ExternalInput")
    maskt_d = nc.dram_tensor("maskt", [128, 4 * QC], f32, kind="ExternalInput")
    ident_d = nc.dram_tensor("ident", [128, 64], f32r, kind="ExternalInput")
    outT_d = nc.dram_tensor("outT", [C, TPC], f32r, kind="ExternalOutput")
    czT_d = nc.dram_tensor("czT", [C, TPC], f32r, kind="ExternalOutput")

    with tile.TileContext(nc) as tc:
        with tc.tile_pool(name="sb", bufs=1) as sb, \
             tc.tile_pool(name="ps", bufs=1, space="PSUM") as ps, \
             tc.tile_pool(name="dr", bufs=1, space="DRAM") as dr:

            # ---- constants (wqkv on the HWDGE queue ahead of the xT stream;
            #      the rest on SWDGE so they don't delay it) ----
            wqkv_sb = sb.tile([128, 3 * 8 * 128], f32r)     # [c_in 128][(m,kt,c)]
            for mq in range(3):
                nc.gpsimd.dma_start(wqkv_sb[:, mq * 1024:(mq + 1) * 1024],
                                    wqkv_d[:, mq * 1024:(mq + 1) * 1024])
            maskt_sb = sb.tile([128, 4 * QC], f32)
            nc.gpsimd.dma_start(maskt_sb[:], maskt_d[:])
            ident_sb = sb.tile([128, 64], f32r)
            nc.gpsimd.dma_start(ident_sb[:], ident_d[:])
            bias_sb = sb.tile([128, 16], f32)               # cols 0:8 = bp tiles, 8:16 = bc
            nc.gpsimd.dma_start(bias_sb[:], bias_d[:])
            ones64 = sb.tile([1, 64], f32r)
            nc.vector.tensor_scalar(ones64[:], maskt_sb[0:1, 0:64], 0.0, 1.0, ALU.mult, ALU.add)

            for _rep in range(reps):
                qT_sb = sb.tile([128, TOK], f32r, tag="qT", bufs=1, name="qT_sb")
                kT_sb = sb.tile([128, TOK], f32r, tag="kT", bufs=1, name="kT_sb")
                # rotating 16KB scratch: vT -> outT -> otf -> pj -> cz
                vT_sb = sb.tile([128, TOK], f32r, tag="scratch16", bufs=2, name="vT_sb")
                v_aug = sb.tile([128, B * 2 * NKT * 65], f32r, tag="v_aug", bufs=1, name="v_aug")
                outT_sb = sb.tile([128, TOK], f32r, tag="scratch16", bufs=2, name="outT_sb")
                # ones columns of v_aug, written once up front
                nc.vector.tensor_scalar(v_aug[:, 64::65], maskt_sb[:, 0:B * 2 * NKT],
                                        0.0, 1.0, ALU.mult, ALU.add)

                wpms, wcms = {}, {}
                a2a_in = dr.tile([NCORES, 128, TPC], f32r, name="a2a_in")
                a2a_out = dr.tile([NCORES, 128, TPC], f32r, name="a2a_out")
                # ---- fused pipeline over token chunks: QKV(n) -> vT(n) -> attention(b,j=n) ----
                for n in range(TOK // QC):
                    b, j = n // NQC, n % NQC
                    # QKV for chunk n, kt-outer so the xT stream overlaps compute
                    accs = [ps.tile([128, QC], f32, tag="mm", bufs=3, name=f"qkv_{n}_{m}")
                            for m in range(3)]
                    for kt in range(8):
                        xt = sb.tile([128, QC], f32r, tag="xt", bufs=10, name=f"xt_{n}_{kt}")
                        dma_eng = nc.sync if kt % 2 == 0 else nc.scalar
                        dma_eng.dma_start(xt[:], xT_d[kt * 128:(kt + 1) * 128, n * QC:(n + 1) * QC])
                        for m in range(3):
                            nc.tensor.matmul(accs[m][:],
                                             wqkv_sb[:, (m * 8 + kt) * 128:(m * 8 + kt + 1) * 128],
                                             xt[:], start=(kt == 0), stop=(kt == 7))
                    nc.scalar.activation(qT_sb[:, n * QC:(n + 1) * QC], accs[0][:], AF.Copy)
                    nc.vector.tensor_copy(kT_sb[:, n * QC:(n + 1) * QC], accs[1][:])
                    nc.vector.tensor_copy(vT_sb[:, n * QC:(n + 1) * QC], accs[2][:])
                    # V transposes for this chunk
                    for h in range(2):
                        for kt in range(4 * j, 4 * j + 4):
                            tp = ps.tile([128, 64], f32r, tag="mm", bufs=3, name=f"vt_{n}_{h}_{kt}")
                            nc.tensor.transpose(
                                tp[:], vT_sb[h * 64:(h + 1) * 64,
                                             b * N + kt * 128:b * N + (kt + 1) * 128],
                                ident_sb[h * 64:(h + 1) * 64, :])
                            slot = ((b * 2 + h) * NKT + kt) * 65
                            nc.vector.tensor_copy(v_aug[:, slot:slot + 64], tp[:])
                    if phase < 1:
                        continue
                    # attention for (b, j): heads sequential; AV trails scores by one
                    # group so PE never in-order-blocks on the exp it needs
                    qc0 = b * N + j * QC
                    last_kt = 4 * j + 3
                    for h in range(2):
                        av = ps.tile([65, QC], f32, tag="av", bufs=1, name=f"av_{n}_{h}")
                        ngroups = 2 * j + 2
                        exs = {}
                        for gi in range(ngroups + 1):
                            if gi < ngroups:
                                kt0 = 2 * gi
                                sc2 = ps.tile([128, 2 * QC], f32, tag="sc", bufs=2,
                                              name=f"sc_{n}_{gi}_{h}")
                                for t in range(2):
                                    kk = b * N + (kt0 + t) * 128
                                    nc.tensor.matmul(sc2[:, t * QC:(t + 1) * QC],
                                                     kT_sb[h * 64:(h + 1) * 64, kk:kk + 128],
                                                     qT_sb[h * 64:(h + 1) * 64, qc0:qc0 + QC],
                                                     start=True, stop=True)
                                ex2 = sb.tile([128, 2 * QC], f32r, tag="ex", bufs=6,
                                              name=f"ex_{n}_{gi}_{h}")
                                nc.scalar.activation(ex2[:], sc2[:], AF.Exp, scale=HD ** -0.5)
                                if kt0 >= 4 * j:
                                    off = (kt0 - 4 * j) * QC
                                    nc.vector.tensor_tensor(ex2[:], ex2[:],
                                                            maskt_sb[:, off:off + 2 * QC], ALU.mult)
                                exs[gi] = ex2
                            gav = gi - 1 if av_delay else gi
                            if 0 <= gav < ngroups and (av_delay or gi < ngroups):
                                exa = exs.pop(gav)
                                for t in range(2):
                                    kt = 2 * gav + t
                                    slot = ((b * 2 + h) * NKT + kt) * 65
                                    nc.tensor.matmul(av[:], v_aug[:, slot:slot + 65],
                                                     exa[:, t * QC:(t + 1) * QC],
                                                     start=(kt == 0), stop=(kt == last_kt))
                        rc = sb.tile([1, QC], f32r, tag="rc", bufs=2, name=f"rc_{n}_{h}")
                        with nc.allow_low_precision(reason="softmax denom reciprocal"):
                            nc.vector.reciprocal(rc[:], av[64:65, :])
                        if pe_norm:
                            sclp = ps.tile([64, QC], f32, tag="mm", bufs=3, name=f"sclp_{n}_{h}")
                            nc.tensor.matmul(sclp[:], ones64[:], rc[:], start=True, stop=True)
                            rawv = sb.tile([64, QC], f32, tag="rawv", bufs=2, name=f"rawv_{n}_{h}")
                            nc.vector.tensor_copy(rawv[:], av[0:64, :])
                            nc.vector.tensor_tensor(outT_sb[h * 64:(h + 1) * 64, qc0:qc0 + QC],
                                                    rawv[:], sclp[:], ALU.mult)
                        else:
                            rb = sb.tile([64, QC], f32r, tag="rb", bufs=2, name=f"rb_{n}_{h}")
                            nc.gpsimd.partition_broadcast(rb[:], rc[:])
                            nc.vector.tensor_tensor(outT_sb[h * 64:(h + 1) * 64, qc0:qc0 + QC],
                                                    av[0:64, :], rb[:], ALU.mult)
                    if phase >= 2:
                        # stage this chunk's A2A slice immediately (chunk n == dest core n)
                        nc.gpsimd.dma_start(a2a_in[n], outT_sb[:, n * TPC:(n + 1) * TPC])
                        if n == 5:
                            # prefetch proj/causal weights under the attention tail
                            for m in range(8):
                                wpm = sb.tile([128, 1024], f32r, tag="wp", bufs=8, name=f"wpm_{m}")
                                nc.gpsimd.dma_start(wpm[:], wp_d[m])
                                wpms[m] = wpm
                            for m in range(4):
                                wcm = sb.tile([128, 1024], f32r, tag="wc", bufs=4, name=f"wcm_{m}")
                                nc.gpsimd.dma_start(wcm[:], wc_d[m])
                                wcms[m] = wcm

                if phase < 2:
                    continue
                # ---- AllToAll: head-parallel -> token-parallel (slices staged per chunk) ----
                if fake_a2a:
                    nc.sync.dma_start(a2a_out[:], a2a_in[:])
                else:
                    nc.gpsimd.collective_compute("AllToAll", ALU.bypass,
                                                 replica_groups=[list(range(NCORES))],
                                                 ins=[a2a_in.opt()], outs=[a2a_out.opt()])
                otf = sb.tile([128, NCORES * TPC], f32r, tag="scratch16", bufs=2, name="otf")
                for d in range(NCORES):
                    nc.sync.dma_start(otf[:, d * TPC:(d + 1) * TPC], a2a_out[d])

                if phase < 3:
                    continue
                # ---- proj + bias (projT [c_out, tok]) ----
                pj_sb = sb.tile([128, C // 128 * TPC], f32r, tag="scratch16", bufs=2, name="pj_sb")
                for m in range(8):
                    wpm = wpms[m]
                    acc = ps.tile([128, TPC], f32, tag="sc", bufs=2, name=f"pj_ps_{m}")
                    for kt in range(8):
                        nc.tensor.matmul(acc[:], wpm[:, kt * 128:(kt + 1) * 128],
                                         otf[:, kt * TPC:(kt + 1) * TPC],
                                         start=(kt == 0), stop=(kt == 7))
                    nc.vector.tensor_scalar_add(pj_sb[:, m * TPC:(m + 1) * TPC], acc[:],
                                                bias_sb[:, m:m + 1])
                    nc.sync.dma_start(outT_d[m * 128:(m + 1) * 128, :], pj_sb[:, m * TPC:(m + 1) * TPC])

                # ---- causal_features + bias ----
                cz_sb = sb.tile([128, C // 128 * TPC], f32r, tag="scratch16", bufs=2, name="cz_sb")
                for m in range(8):
                    if m in wcms:
                        wcm = wcms[m]
                    else:
                        wcm = sb.tile([128, 1024], f32r, tag="wc", bufs=4, name=f"wcm_{m}")
                        nc.gpsimd.dma_start(wcm[:], wc_d[m])
                    acc = ps.tile([128, TPC], f32, tag="sc", bufs=2, name=f"cz_ps_{m}")
                    for kt in range(8):
                        nc.tensor.matmul(acc[:], wcm[:, kt * 128:(kt + 1) * 128],
                                         pj_sb[:, kt * TPC:(kt + 1) * TPC],
                                         start=(kt == 0), stop=(kt == 7))
                    nc.vector.tensor_scalar_add(cz_sb[:, m * TPC:(m + 1) * TPC], acc[:],
                                                bias_sb[:, 8 + m:9 + m])
                    nc.sync.dma_start(czT_d[m * 128:(m + 1) * 128, :], cz_sb[:, m * TPC:(m + 1) * TPC])

    nc.finalize()
    return nc


def _pack_w(w):
    # [kt*128+p, m*128+c] -> [m, p, kt*128+c] contiguous per m-slice
    w = np.asarray(w, dtype=np.float32).reshape(8, 128, 8, 128)
    return np.ascontiguousarray(w.transpose(2, 1, 0, 3).reshape(8, 128, 1024))


def _host_inputs(x, mask, W_qkv, W_proj, b_proj, W_causal, b_causal):
    x = np.asarray(x, dtype=np.float32)
    xT = np.ascontiguousarray(x.reshape(TOK, C).T)
    m2 = np.asarray(mask).reshape(N, N)
    # diagonal-band mask multiplier tiles in S^T [k, q] layout, offsets d0 = k0-q0
    q0 = N - QC
    tiles = []
    for d0 in (0, 128, 256, 384):
        k0 = q0 + d0
        tiles.append(np.ascontiguousarray(m2[q0:q0 + QC, k0:k0 + 128].T.astype(np.float32)))
    maskt = np.concatenate(tiles, axis=1)
    ident = np.ascontiguousarray(np.concatenate([np.eye(64, dtype=np.float32)] * 2, axis=0))
    W_qkv = np.asarray(W_qkv, dtype=np.float32)
    shared = {
        "xT": xT,
        "wp": _pack_w(W_proj),
        "wc": _pack_w(W_causal),
        "bias": np.ascontiguousarray(np.stack(
            [np.asarray(b_proj, np.float32).reshape(8, 128),
             np.asarray(b_causal, np.float32).reshape(8, 128)]).transpose(2, 0, 1).reshape(128, 16)),
        "maskt": maskt,
        "ident": ident,
    }
    in_maps = []
    for i in range(NCORES):
        im = dict(shared)
        sl = np.stack([W_qkv[:, m * C + i * 128:m * C + (i + 1) * 128] for m in range(3)])
        # [m, kt*128+p, c] -> [p, m, kt, c]
        sl = sl.reshape(3, 8, 128, 128).transpose(2, 0, 1, 3).reshape(128, 3 * 8 * 128)
        im["wqkv"] = np.ascontiguousarray(sl)
        in_maps.append(im)
    return in_maps


def kernel(x, mask, W_qkv, W_proj, b_proj, W_causal, b_causal):
    if "nc" not in _CACHE:
        _CACHE["nc"] = _build_program()
    nc = _CACHE["nc"]
    in_maps = _host_inputs(x, mask, W_qkv, W_proj, b_proj, W_causal, b_causal)
    res = run_bass_kernel_spmd(nc, in_maps, list(range(NCORES)))
    out = np.empty((TOK, C), dtype=np.float32)
    cz = np.empty((TOK, C), dtype=np.float32)
    for i in range(NCORES):
        out[i * TPC:(i + 1) * TPC, :] = res.results[i]["outT"].T
        cz[i * TPC:(i + 1) * TPC, :] = res.results[i]["czT"].T
    return (out.reshape(B, N, C), cz.reshape(B, N, C))



# revision 21
# speedup vs baseline: 1.1928x; 1.1928x over previous
"""Trainium2 Bass kernel for causal attention + proj + causal_features.

Problem shapes: x [2, 2048, 1024], H=16 heads, HD=64.
Strategy (8 NeuronCores):
  - Head-parallel attention: core i computes QKV + attention for heads {2i, 2i+1}
    over both batches, everything in transposed [channel, token] layout so the
    contraction dim sits on SBUF partitions.
  - Scores computed as S^T[k, q] = K @ Q^T (contraction d=64); exp on ScalarE
    (scale=1/8 folded in); causal mask applied only on diagonal-band tiles via
    a compact shifted 0/1 band mask; upper-triangle tiles skipped entirely and
    diagonal-band tiles narrowed to the live q-range (min 256 wide to keep the
    f32r 1-cycle/row matmul rate).
  - attn @ V via lhsT = [V | 1] (ones column) so the softmax denominators fall
    out of the same matmul as row 64 of the PSUM accumulator.
  - Two AllToAlls reshard attention output from head-parallel to
    token-parallel, split by head so A2A#0 (head-even rows) overlaps the whole
    head-odd attention pass: per chunk n we run QKV(n) + attention(n, h=0),
    then A2A#0, then attention(*, h=1), then A2A#1, then proj + causal for the
    core's 512-token slice. Outputs returned transposed and re-assembled on
    host.
  - DMAs are merged into few large transfers (HWDGE setup is ~0.6us each):
    x arrives as [128, kt, tok] so half-chunks load in one descriptor set,
    proj/causal weights as [128, m*1024+...] in one DMA apiece.
  - All matmuls run in float32r (full PE rate at free dim >= 256, ~1e-4 rel err).
"""

import numpy as np
import ml_dtypes  # noqa: F401  (registers bfloat16 with numpy)
import concourse.bass as bass
import concourse.mybir as mybir
import concourse.tile as tile
from concourse import bacc
from concourse.bass_utils import run_bass_kernel_spmd

B, N, C, H, HD = 2, 2048, 1024, 16, 64
NCORES = 8
TOK = B * N            # 4096 global tokens
TPC = TOK // NCORES    # 512 tokens per core (output slice)
QC = 512               # q chunk width
KTILE = 128            # k tile height
NKT = N // KTILE       # 16 k tiles per batch
NQC = N // QC          # 4 q chunks per batch
OFF = (0, 128, 256, 256)  # q-narrowing offset per diagonal-band tile (ap>=256)
f32 = mybir.dt.float32
f32r = mybir.dt.float32r
AF = mybir.ActivationFunctionType
ALU = mybir.AluOpType

_CACHE = {}


def _build_program(fake_a2a=False, reps=1, phase=99, nsplit=2):
    nc = bacc.Bacc("TRN2", target_bir_lowering=False, debug=False, num_devices=NCORES)

    xT_d = nc.dram_tensor("xT", [128, 8, TOK], f32r, kind="ExternalInput")
    wqkv_d = nc.dram_tensor("wqkv", [128, 3 * 8 * 128], f32r, kind="ExternalInput")
    wp_d = nc.dram_tensor("wp", [128, 8 * 1024], f32r, kind="ExternalInput")
    wc_d = nc.dram_tensor("wc", [128, 8 * 1024], f32r, kind="ExternalInput")
    bias_d = nc.dram_tensor("bias", [128, 16], f32, kind="ExternalInput")
    mbuf_d = nc.dram_tensor("mbuf", [128, 640], f32, kind="ExternalInput")
    ident_d = nc.dram_tensor("ident", [128, 64], f32r, kind="ExternalInput")
    outT_d = nc.dram_tensor("outT", [128, 8 * TPC], f32r, kind="ExternalOutput")
    czT_d = nc.dram_tensor("czT", [128, 8 * TPC], f32r, kind="ExternalOutput")

    with tile.TileContext(nc) as tc:
        with tc.tile_pool(name="sb", bufs=1) as sb, \
             tc.tile_pool(name="ps", bufs=1, space="PSUM") as ps, \
             tc.tile_pool(name="dr", bufs=1, space="DRAM") as dr:

            # ---- constants: wqkv kt0 slices first (unblock chunk 0); the
            #      kt1-7 remainder + small consts are interleaved behind chunk
            #      0's first two x pieces (see qkv_chunk) ----
            wqkv_sb = sb.tile([128, 3 * 8 * 128], f32r)
            QUEUES = (nc.gpsimd, nc.sync, nc.scalar)
            for m in range(3):
                c0 = m * 1024
                QUEUES[m].dma_start(wqkv_sb[:, c0:c0 + 128], wqkv_d[:, c0:c0 + 128])
            ident_sb = sb.tile([128, 64], f32r)
            mbuf_sb = sb.tile([128, 640], f32)
            bias_sb = sb.tile([128, 16], f32)   # cols 0:8 = bp tiles, 8:16 = bc

            def finish_consts():
                for m in range(3):
                    c0 = m * 1024 + 128
                    QUEUES[m].dma_start(wqkv_sb[:, c0:c0 + 896], wqkv_d[:, c0:c0 + 896])
                nc.gpsimd.dma_start(ident_sb[:], ident_d[:])
                nc.gpsimd.dma_start(mbuf_sb[:], mbuf_d[:])
                nc.gpsimd.dma_start(bias_sb[:], bias_d[:])

            for _rep in range(reps):
                qT_sb = sb.tile([128, TOK], f32r, tag="qT", bufs=1, name="qT_sb")
                kT_sb = sb.tile([128, TOK], f32r, tag="kT", bufs=1, name="kT_sb")
                # rotating 16KB scratch: vT -> otf -> pj -> cz  (+ outT slot)
                vT_sb = sb.tile([128, TOK], f32r, tag="scratch16", bufs=2, name="vT_sb")
                v_aug = sb.tile([128, B * 2 * NKT * 65], f32r, tag="v_aug", bufs=1, name="v_aug")
                outT_sb = sb.tile([128, TOK], f32r, tag="scratch16", bufs=2, name="outT_sb")
                nc.vector.tensor_scalar(v_aug[:, 64::65], mbuf_sb[:, 0:64], 0.0, 1.0,
                                        ALU.mult, ALU.add)

                a2a_in = [dr.tile([NCORES, 64, TPC], f32r, name=f"a2a_in{h}")
                          for h in range(2)]
                a2a_out = [dr.tile([NCORES, 64, TPC], f32r, name=f"a2a_out{h}")
                           for h in range(2)]

                def qkv_chunk(n):
                    accs = [ps.tile([128, QC], f32, tag="big", bufs=3,
                                    name=f"qkv_{n}_{m}") for m in range(3)]
                    nspl = 8 if n == 0 else 2
                    kspan = 8 // nspl
                    xts = []
                    for s in range(nspl):
                        xt = sb.tile([128, 4, QC], f32r, tag="xt", bufs=4,
                                     name=f"xt_{n}_{s}")
                        dma_eng = nc.sync if (n * nspl + s) % 2 == 0 else nc.scalar
                        dma_eng.dma_start(xt[:, 0:kspan, :],
                                          xT_d[:, s * kspan:(s + 1) * kspan,
                                               n * QC:(n + 1) * QC])
                        xts.append(xt)
                        if n == 0 and s == 1 and _rep == 0:
                            finish_consts()
                    for kt in range(8):
                        rhs = xts[kt // kspan][:, kt % kspan, :]
                        for m in range(3):
                            nc.tensor.matmul(
                                accs[m][:],
                                wqkv_sb[:, (m * 8 + kt) * 128:(m * 8 + kt + 1) * 128],
                                rhs, start=(kt == 0), stop=(kt == 7))
                    nc.scalar.activation(qT_sb[:, n * QC:(n + 1) * QC], accs[0][:], AF.Copy)
                    nc.vector.tensor_copy(kT_sb[:, n * QC:(n + 1) * QC], accs[1][:])
                    nc.scalar.activation(vT_sb[:, n * QC:(n + 1) * QC], accs[2][:], AF.Copy)

                def transposes(n):
                    b, j = n // NQC, n % NQC
                    for h in range(2):
                        for kt in range(4 * j, 4 * j + 4):
                            tp = ps.tile([128, 64], f32r, tag="big", bufs=3,
                                         name=f"vt_{n}_{h}_{kt}")
                            nc.tensor.transpose(
                                tp[:], vT_sb[h * 64:(h + 1) * 64,
                                             b * N + kt * 128:b * N + (kt + 1) * 128],
                                ident_sb[h * 64:(h + 1) * 64, :])
                            slot = ((b * 2 + h) * NKT + kt) * 65
                            nc.vector.tensor_copy(v_aug[:, slot:slot + 64], tp[:])

                def attention(n, h, emit_tp=False):
                    b, j = n // NQC, n % NQC
                    qc0 = b * N + j * QC
                    last_kt = 4 * j + 3
                    if emit_tp and j == 0:
                        transposes(n)
                    av = ps.tile([65, QC], f32, tag="av", bufs=2, name=f"av_{n}_{h}")
                    for gi in range(2 * j + 2):
                        kt0 = 2 * gi
                        band = kt0 >= 4 * j
                        sc2 = ps.tile([128, 2 * QC], f32, tag="big", bufs=3,
                                      name=f"sc_{n}_{gi}_{h}")
                        ex2 = sb.tile([128, 2 * QC], f32r, tag="ex", bufs=3,
                                      name=f"ex_{n}_{gi}_{h}")
                        for t in range(2):
                            kt = kt0 + t
                            off = OFF[kt - 4 * j] if band else 0
                            kk = b * N + kt * 128
                            nc.tensor.matmul(sc2[:, t * QC + off:(t + 1) * QC],
                                             kT_sb[h * 64:(h + 1) * 64, kk:kk + 128],
                                             qT_sb[h * 64:(h + 1) * 64, qc0 + off:qc0 + QC],
                                             start=True, stop=True)
                        if emit_tp and j > 0 and gi == 0:
                            transposes(n)
                        if band:
                            for t in range(2):
                                d = kt0 + t - 4 * j
                                off = OFF[d]
                                nc.scalar.activation(ex2[:, t * QC + off:(t + 1) * QC],
                                                     sc2[:, t * QC + off:(t + 1) * QC],
                                                     AF.Exp, scale=HD ** -0.5)
                                nc.vector.tensor_tensor(
                                    ex2[:, t * QC + off:(t + 1) * QC],
                                    ex2[:, t * QC + off:(t + 1) * QC],
                                    mbuf_sb[:, 128 - 128 * d + off:640 - 128 * d],
                                    ALU.mult)
                        else:
                            nc.scalar.activation(ex2[:], sc2[:], AF.Exp, scale=HD ** -0.5)
                        for t in range(2):
                            kt = kt0 + t
                            off = OFF[kt - 4 * j] if band else 0
                            slot = ((b * 2 + h) * NKT + kt) * 65
                            nc.tensor.matmul(av[:, off:QC], v_aug[:, slot:slot + 65],
                                             ex2[:, t * QC + off:(t + 1) * QC],
                                             start=(kt == 0), stop=(kt == last_kt),
                                             skip_group_check=(off > 0))
                    rc = sb.tile([1, QC], f32r, tag="rc", bufs=1, name=f"rc_{n}_{h}")
                    with nc.allow_low_precision(reason="softmax denom reciprocal"):
                        nc.vector.reciprocal(rc[:], av[64:65, :])
                    rb = sb.tile([64, QC], f32r, tag="rb", bufs=1, name=f"rb_{n}_{h}")
                    nc.gpsimd.partition_broadcast(rb[:], rc[:])
                    nc.vector.tensor_tensor(outT_sb[h * 64:(h + 1) * 64, qc0:qc0 + QC],
                                            av[0:64, :], rb[:], ALU.mult)

                def a2a(h):
                    if fake_a2a:
                        nc.gpsimd.dma_start(a2a_out[h][:], a2a_in[h][:])
                    else:
                        nc.gpsimd.collective_compute(
                            "AllToAll", ALU.bypass,
                            replica_groups=[list(range(NCORES))],
                            ins=[a2a_in[h].opt()], outs=[a2a_out[h].opt()])

                # ---- phase B1: QKV + head-0 attention, chunk-fused; the light
                #      (small-j) chunks also run head-1 inline to fill the
                #      DMA-starved windows ----
                EARLY_H1 = (0, 1, 4, 5)
                for n in range(8):
                    qkv_chunk(n)
                    if phase >= 1:
                        attention(n, 0, emit_tp=True)
                        if phase >= 2 and nsplit == 2:
                            nc.gpsimd.dma_start(a2a_in[0][n],
                                                outT_sb[0:64, n * TPC:(n + 1) * TPC])
                        if n in EARLY_H1:
                            attention(n, 1)
                            if phase >= 2:
                                nc.gpsimd.dma_start(a2a_in[1][n],
                                                    outT_sb[64:128, n * TPC:(n + 1) * TPC])
                    else:
                        transposes(n)
                if phase < 1:
                    continue
                if phase >= 2 and nsplit == 2:
                    a2a(0)

                # ---- phase B2: head-1 attention; A2A#0 + weight loads overlap ----
                wpm = sb.tile([128, 8 * 1024], f32r, tag="wp", bufs=1, name="wpm")
                nc.sync.dma_start(wpm[:], wp_d[:])
                wcm = sb.tile([128, 8 * 1024], f32r, tag="wc", bufs=1, name="wcm")
                nc.sync.dma_start(wcm[:], wc_d[:])
                otf = sb.tile([128, NCORES * TPC], f32r, tag="scratch16", bufs=2, name="otf")
                if phase >= 2 and nsplit == 2:
                    for d in range(NCORES):
                        nc.sync.dma_start(otf[0:64, d * TPC:(d + 1) * TPC], a2a_out[0][d])
                for n in range(8):
                    if n in EARLY_H1:
                        continue
                    attention(n, 1)
                    if phase >= 2:
                        nc.gpsimd.dma_start(a2a_in[1][n],
                                            outT_sb[64:128, n * TPC:(n + 1) * TPC])
                if phase < 2:
                    continue
                if nsplit == 1:
                    for nn in range(8):
                        nc.gpsimd.dma_start(a2a_in[0][nn],
                                            outT_sb[0:64, nn * TPC:(nn + 1) * TPC])
                    a2a(0)
                    for d in range(NCORES):
                        nc.sync.dma_start(otf[0:64, d * TPC:(d + 1) * TPC], a2a_out[0][d])
                a2a(1)
                for d in range(NCORES):
                    eng = (nc.sync, nc.scalar, nc.gpsimd)[d % 3]
                    eng.dma_start(otf[64:128, d * TPC:(d + 1) * TPC], a2a_out[1][d])

                if phase < 3:
                    continue
                # ---- proj + bias (projT [c_out, tok]) ----
                pj_sb = sb.tile([128, C // 128 * TPC], f32r, tag="scratch16", bufs=2, name="pj_sb")
                for m in range(8):
                    acc = ps.tile([128, TPC], f32, tag="av", bufs=2, name=f"pj_ps_{m}")
                    for kt in range(8):
                        nc.tensor.matmul(acc[:], wpm[:, m * 1024 + kt * 128:m * 1024 + (kt + 1) * 128],
                                         otf[:, kt * TPC:(kt + 1) * TPC],
                                         start=(kt == 0), stop=(kt == 7))
                    nc.vector.tensor_scalar_add(pj_sb[:, m * TPC:(m + 1) * TPC], acc[:],
                                                bias_sb[:, m:m + 1])
                    if m % 4 == 3:
                        nc.sync.dma_start(outT_d[:, (m - 3) * TPC:(m + 1) * TPC],
                                          pj_sb[:, (m - 3) * TPC:(m + 1) * TPC])

                # ---- causal_features + bias ----
                cz_sb = sb.tile([128, C // 128 * TPC], f32r, tag="scratch16", bufs=2, name="cz_sb")
                for m in range(8):
                    acc = ps.tile([128, TPC], f32, tag="av", bufs=2, name=f"cz_ps_{m}")
                    for kt in range(8):
                        nc.tensor.matmul(acc[:], wcm[:, m * 1024 + kt * 128:m * 1024 + (kt + 1) * 128],
                                         pj_sb[:, kt * TPC:(kt + 1) * TPC],
                                         start=(kt == 0), stop=(kt == 7))
                    nc.vector.tensor_scalar_add(cz_sb[:, m * TPC:(m + 1) * TPC], acc[:],
                                                bias_sb[:, 8 + m:9 + m])
                    if m % 4 == 3:
                        nc.sync.dma_start(czT_d[:, (m - 3) * TPC:(m + 1) * TPC],
                                          cz_sb[:, (m - 3) * TPC:(m + 1) * TPC])

    nc.finalize()
    return nc


def _pack_w(w):
    # [kt*128+p, m*128+c] -> [p, m, kt*128+c] flattened to [128, 8192]
    w = np.asarray(w, dtype=np.float32).reshape(8, 128, 8, 128)
    return np.ascontiguousarray(w.transpose(1, 2, 0, 3).reshape(128, 8 * 1024))


def _host_inputs(x, mask, W_qkv, W_proj, b_proj, W_causal, b_causal):
    x = np.asarray(x, dtype=np.float32)
    # xT[p, kt, tok] = x[tok, kt*128 + p]
    xT = np.ascontiguousarray(x.reshape(TOK, 8, 128).transpose(2, 1, 0))
    m2 = np.asarray(mask).reshape(N, N)
    # compact diagonal-band mask: mbuf[k, c] = 1 iff c >= k + 384, so band
    # tile d (k offset 128*d above the q chunk) is mbuf[:, 384-128d : 896-128d]
    q0 = N - QC
    mbuf = np.zeros((128, 640), dtype=np.float32)
    mbuf[:, 128:] = m2[q0:q0 + QC, q0:q0 + 128].T.astype(np.float32)
    ident = np.ascontiguousarray(np.concatenate([np.eye(64, dtype=np.float32)] * 2, axis=0))
    W_qkv = np.asarray(W_qkv, dtype=np.float32)
    shared = {
        "xT": xT,
        "wp": _pack_w(W_proj),
        "wc": _pack_w(W_causal),
        "bias": np.ascontiguousarray(np.stack(
            [np.asarray(b_proj, np.float32).reshape(8, 128),
             np.asarray(b_causal, np.float32).reshape(8, 128)]).transpose(2, 0, 1).reshape(128, 16)),
        "mbuf": mbuf,
        "ident": ident,
    }
    in_maps = []
    for i in range(NCORES):
        im = dict(shared)
        sl = np.stack([W_qkv[:, m * C + i * 128:m * C + (i + 1) * 128] for m in range(3)])
        # [m, kt*128+p, c] -> [p, m, kt, c]
        sl = sl.reshape(3, 8, 128, 128).transpose(2, 0, 1, 3).reshape(128, 3 * 8 * 128)
        im["wqkv"] = np.ascontiguousarray(sl)
        in_maps.append(im)
    return in_maps


def _unpackT(arr):
    # [p, m*512 + tok] -> [tok, m*128 + p]
    return arr.reshape(128, 8, TPC).transpose(2, 1, 0).reshape(TPC, C)


def kernel(x, mask, W_qkv, W_proj, b_proj, W_causal, b_causal):
    if "nc" not in _CACHE:
        _CACHE["nc"] = _build_program()
    nc = _CACHE["nc"]
    in_maps = _host_inputs(x, mask, W_qkv, W_proj, b_proj, W_causal, b_causal)
    res = run_bass_kernel_spmd(nc, in_maps, list(range(NCORES)))
    out = np.empty((TOK, C), dtype=np.float32)
    cz = np.empty((TOK, C), dtype=np.float32)
    for i in range(NCORES):
        out[i * TPC:(i + 1) * TPC, :] = _unpackT(res.results[i]["outT"])
        cz[i * TPC:(i + 1) * TPC, :] = _unpackT(res.results[i]["czT"])
    return (out.reshape(B, N, C), cz.reshape(B, N, C))
